# revision 65
# baseline (speedup 1.0000x reference)
"""Trainium2 Bass kernel for a transformer block (LN1->MHA->+res->LN2->FFN->+res).

Sharding: data-parallel over batch. B=8 batch elements == 8 NeuronCores; each
core runs the whole block for one batch element (no collectives).

Per-core dataflow (T=1024, D=1024, 16 heads x 64), tuned for PE clock (HAM)
warmth -- the PE runs at 1.2 GHz until it has been busy for a full ~3.4us
window, then 2.4 GHz; every phase keeps a dense back-to-back matmul stream:
  - ~9 dummy warmup matmuls at kernel start flip the HAM throttle to 8/8
    before the first real QKV matmul
  - LN1 token-major per 128-token chunk (ACT accum_out stats); gamma/beta
    folded into the transpose-evacuation tensor_scalar; activations then
    FEATURE-major [D, T] (f32r) through the dense chain
  - causal attention per head: scoresT[s,t] block-rows land in a single
    2-bank PSUM tile [128,1024]; ONE Exp ACT per (head, s-block) covers the
    whole causal range (fuses 1/8 scale + f32->bf16); score matmuls are
    causally tightened; diagonal masked post-exp on GpSimd
  - softmax denominator from a ones-column in v (attn@[v|1]); per-partition
    reciprocal + tensor_scalar divide on DVE
  - projection in bf16 (weights streamed bf16); LN2 stats (ones-matmuls)
    interleaved with the projection matmuls, row sums copied to SBUF at proj
    end; LN2 row ops + per-feature apply feed both an f32 token-major
    residual ledger (xp2, +b2) and a bf16 copy (xb2) for ff1
  - FFN: w1/w2 streamed bf16 (w1 first set prefetched during attention,
    rotating through a 10-deep pool); relu+bias evac on the (idle) Scalar
    engine; ff2 partial sums accumulate straight into xp2 on DVE
"""

import sys

sys.path.insert(0, "/opt/trn_rl_repo")

import numpy as np
import ml_dtypes

import concourse.bacc as bacc
import concourse.mybir as mybir
from concourse.tile import TileContext
from concourse import bass_utils

F32 = mybir.dt.float32
F32R = mybir.dt.float32r
BF16 = mybir.dt.bfloat16
FP8 = mybir.dt.float8e4
AF = mybir.ActivationFunctionType
ALU = mybir.AluOpType
DR = mybir.MatmulPerfMode.DoubleRow

# xln1 is carried at RSCL x its true value (folded into gamma1) so the fp8
# proj output (attn x32 ones-fold, w_proj x64) lands on the same scale as
# the residual with zero extra ops; LN2 is scale-invariant modulo eps.
RSCL = 2048.0

B, T, D = 8, 1024, 1024
H, E = 16, 64
DFF = 4 * D
NCORE = 8
NT = T // 128
ND = D // 128
NJ = DFF // 128
LN_EPS = 1e-5


class _Done(Exception):
    pass


def r32(ap):
    return ap.bitcast(F32R)


def v32(ap):
    return ap.bitcast(F32)


def _build(upto=9):
    nc = bacc.Bacc("TRN2", target_bir_lowering=False, debug=False,
                   num_devices=NCORE)

    x_l = nc.dram_tensor("x_l", [T, D], F32R, kind="ExternalInput")
    wq8 = nc.dram_tensor("wq8", [H // 2, 128, 1024], FP8,
                         kind="ExternalInput")
    wk8 = nc.dram_tensor("wk8", [H // 2, 128, 1024], FP8,
                         kind="ExternalInput")
    wv8 = nc.dram_tensor("wv8", [ND // 2, 128, 2048], FP8,
                         kind="ExternalInput")
    wpj8 = nc.dram_tensor("wpj8", [ND // 2, 128, 2048], FP8,
                          kind="ExternalInput")
    w1r = nc.dram_tensor("w1r", [NJ, 128, ND, 128], BF16,
                         kind="ExternalInput")
    w2b = nc.dram_tensor("w2b", [DFF, D], BF16, kind="ExternalInput")
    g1f = nc.dram_tensor("g1f", [128, ND], F32, kind="ExternalInput")
    be1f = nc.dram_tensor("be1f", [128, ND], F32, kind="ExternalInput")
    g2f = nc.dram_tensor("g2f", [128, ND], F32, kind="ExternalInput")
    be2f = nc.dram_tensor("be2f", [128, ND], F32, kind="ExternalInput")
    bpf = nc.dram_tensor("bpf", [128, ND], F32, kind="ExternalInput")
    b1f = nc.dram_tensor("b1f", [128, NJ], F32, kind="ExternalInput")
    b2b = nc.dram_tensor("b2b", [128, D], F32, kind="ExternalInput")
    idn = nc.dram_tensor("idn", [128, 128], F32R, kind="ExternalInput")
    onz = nc.dram_tensor("onz", [128, 128], F32R, kind="ExternalInput")
    ond = nc.dram_tensor("ond", [128, 1], F32R, kind="ExternalInput")
    mby = nc.dram_tensor("mby", [128, 128], FP8, kind="ExternalInput")
    out_l = nc.dram_tensor("out_l", [T, D], F32, kind="ExternalOutput")

    def dump_fm(tiles, n):
        for c in range(n):
            nc.sync.dma_start(out_l[128 * c:128 * (c + 1), :],
                              v32(tiles[c][:]))

    with TileContext(nc) as tc:
        with tc.tile_pool(name="const", bufs=1) as cp:
          try:
            def cload(name, dram, shape, dtype=F32):
                t = cp.tile(list(shape), dtype, tag=name, name=name)
                nc.sync.dma_start(t[:], dram[:])
                return t

            c_id = cload("idn", idn, [128, 128], F32R)
            c_g1 = cload("g1f", g1f, [128, ND])
            c_be1 = cload("be1f", be1f, [128, ND])
            c_eps = cp.tile([128, 1], F32, tag="eps", name="eps")
            nc.gpsimd.memset(c_eps[:], LN_EPS)
            c_wrm = cp.tile([128, 384], F32, tag="wrm", name="wrm")
            nc.gpsimd.memset(c_wrm[:], 0.0)

            # ---- HAM warmup: ~4us of dummy matmuls so the PE clock is at
            # 2.4 GHz when the first real matmul issues ----
            with tc.tile_pool(name="ps_wrm", bufs=1, space="PSUM") as psw:
                for _ in range(18):
                    pw = psw.tile([128, 384], F32, tag="wrm", bufs=2,
                                  name="pwrm")
                    nc.tensor.matmul(pw[:], c_id[:], r32(c_wrm[:]),
                                     start=True, stop=True)

            with (
                tc.tile_pool(name="w1s", bufs=10) as w1sp,
                tc.tile_pool(name="fmx", bufs=8) as fmx,
                tc.tile_pool(name="rw2", bufs=1) as rw2,
            ):
                # ========== phase 1: LN1 (token-major) ==========
                xf = [fmx.tile([128, T], F32R, tag="xf", name="xf")
                      for _ in range(ND)]
                # fp8 copy of xln1 (true scale) in DoubleRow pair layout
                # [d_in, (jj, par, 512 tok)] for the fp8 QKV matmuls
                x8 = [fmx.tile([128, 2048], FP8, tag="x8", name="x8")
                      for _ in range(ND // 2)]
                with (
                    tc.tile_pool(name="xin", bufs=3) as xp,
                    tc.tile_pool(name="scr", bufs=2) as scrp,
                    tc.tile_pool(name="st1", bufs=4) as st1,
                    tc.tile_pool(name="ps_a", bufs=1, space="PSUM") as psa,
                ):
                    for m in range(NT):
                        xm = xp.tile([128, D], F32R, tag="xin")
                        nc.sync.dma_start(xm[:], x_l[128 * m:128 * (m + 1), :])
                        scr = scrp.tile([128, D], F32, tag="scr", name="scr")
                        st = st1.tile([128, 4], F32, tag="st", name="st")
                        nc.scalar.activation(scr[:], xm[:], AF.Square,
                                             accum_out=st[:, 1:2])
                        nc.scalar.activation(scr[:], xm[:], AF.Identity,
                                             accum_out=st[:, 0:1])
                        nc.vector.tensor_scalar_mul(st[:, 0:1], st[:, 0:1],
                                                    1.0 / D)
                        nc.vector.tensor_scalar_mul(st[:, 1:2], st[:, 1:2],
                                                    1.0 / D)
                        nc.vector.tensor_mul(st[:, 2:3], st[:, 0:1],
                                             st[:, 0:1])
                        nc.vector.tensor_sub(st[:, 2:3], st[:, 1:2],
                                             st[:, 2:3])
                        nc.scalar.activation(st[:, 2:3], st[:, 2:3], AF.Sqrt,
                                             bias=c_eps[:])
                        nc.vector.reciprocal(st[:, 3:4], st[:, 2:3])
                        nc.vector.tensor_scalar(
                            out=xm[:], in0=xm[:], scalar1=st[:, 0:1],
                            scalar2=st[:, 3:4], op0=ALU.subtract, op1=ALU.mult)
                        for c in range(ND):
                            pt = psa.tile([128, 128], F32R, tag="tr",
                                          bufs=3, name="pt")
                            nc.tensor.transpose(
                                pt[:], xm[:, 128 * c:128 * (c + 1)], c_id[:])
                            nc.vector.tensor_scalar(
                                out=xf[c][:, 128 * m:128 * (m + 1)],
                                in0=pt[:], scalar1=c_g1[:, c:c + 1],
                                scalar2=c_be1[:, c:c + 1],
                                op0=ALU.mult, op1=ALU.add)
                # non-critical consts load behind the x chunks
                c_idb = cp.tile([128, 128], BF16, tag="idnb", name="idnb")
                nc.vector.tensor_copy(c_idb[:], c_id[:])
                c_g2 = cload("g2f", g2f, [128, ND])
                c_be2 = cload("be2f", be2f, [128, ND])
                c_bp = cload("bpf", bpf, [128, ND])
                c_b1 = cload("b1f", b1f, [128, NJ])
                c_b2 = cload("b2b", b2b, [128, D])
                c_mb = cload("mby", mby, [128, 128], FP8)
                c_eps2 = cp.tile([128, 1], F32, tag="eps2", name="eps2")
                nc.gpsimd.memset(c_eps2[:], LN_EPS * RSCL * RSCL)
                c_onD = cp.tile([128, 128], F32, tag="onD", name="onD")
                nc.gpsimd.memset(c_onD[:], 1.0 / D)
                # x8 = xf/RSCL on ACT (idle after LN1), into the
                # DoubleRow pair layout (jj, par, tok); jj=0 first so the
                # first q/k matmuls are unblocked early
                for jj in range(2):
                    for c in range(ND):
                        x84 = x8[c // 2].rearrange(
                            "p (jj two n) -> p jj two n", jj=2, two=2)
                        nc.scalar.mul(
                            x84[:, jj, c % 2, :],
                            v32(xf[c][:, 512 * jj:512 * (jj + 1)]),
                            1.0 / RSCL)
                if upto == 1:
                    dump_fm(xf, ND)
                    raise _Done()
                xln1 = xf

                # prefetch FFN w1 set 0 (runs during attention on idle DMA)
                w1q = []
                for j8 in range(8):
                    t = w1sp.tile([128, D], BF16, tag="w1", name="w1t")
                    nc.sync.dma_start(
                        t[:], w1r[j8].rearrange("a b c -> a (b c)"))
                    w1q.append(t)

                with tc.tile_pool(name="fma", bufs=8) as fma:
                  # fp8 attention output in DoubleRow pair layout:
                  # tile cc holds feature blocks (2cc, 2cc+1) side by side
                  ao8 = [fma.tile([128, 2 * T], FP8, tag="ao", name="ao8")
                         for _ in range(ND // 2)]
                  with (
                      tc.tile_pool(name="qk", bufs=24) as qkp,
                      tc.tile_pool(name="vsb", bufs=8) as vp,
                  ):
                    # ========== phase 2: QKV + v (fp8 DoubleRow) ==========
                    qT, kT, v_sb = [], [], []
                    x83 = [x8[cc].rearrange(
                        "p (jj two n) -> p jj two n", jj=2, two=2)
                        for cc in range(ND // 2)]
                    with (
                        tc.tile_pool(name="wq", bufs=2) as wqpool,
                        tc.tile_pool(name="wk", bufs=2) as wkpool,
                        tc.tile_pool(name="ps_q", bufs=1, space="PSUM") as psq,
                    ):
                        for p in range(H // 2):
                            wq_t = wqpool.tile([128, D], FP8, tag="wq")
                            nc.sync.dma_start(
                                wq_t[:], wq8[p])
                            wk_t = wkpool.tile([128, D], FP8, tag="wk")
                            nc.sync.dma_start(
                                wk_t[:], wk8[p])
                            # q is stored TWICE, zero-padded per head, so
                            # the score matmuls can contract all 128 PE rows
                            # (the other head's k sees zeros) -- a lone
                            # 64-row matmul only lights half the array and
                            # the HAM then never grants the full clock
                            qa = qkp.tile([128, T], BF16, tag="qk",
                                          name="qa")
                            qb = qkp.tile([128, T], BF16, tag="qk",
                                          name="qb")
                            k_t = qkp.tile([128, T], BF16, tag="qk",
                                           name="k_t")
                            nc.gpsimd.memset(qa[64:128, :], 0.0)
                            nc.gpsimd.memset(qb[0:64, :], 0.0)
                            for dst, w_t in ((None, wq_t), (k_t, wk_t)):
                                w4 = w_t.rearrange(
                                    "p (cc two m) -> p cc two m",
                                    cc=ND // 2, two=2)
                                for jj in range(2):
                                    sl = slice(512 * jj, 512 * (jj + 1))
                                    ps = psq.tile([128, 512], F32, tag="qk",
                                                  bufs=3, name="psqk")
                                    for cc in range(ND // 2):
                                        nc.tensor.matmul(
                                            ps[:], w4[:, cc, :, :],
                                            x83[cc][:, jj, :, :],
                                            start=(cc == 0),
                                            stop=(cc == ND // 2 - 1),
                                            perf_mode=DR)
                                    if dst is None:
                                        nc.vector.tensor_copy(
                                            qa[0:64, sl], ps[0:64, :])
                                        nc.vector.tensor_copy(
                                            qb[64:128, sl], ps[64:128, :])
                                    else:
                                        nc.vector.tensor_copy(dst[:, sl],
                                                              ps[:])
                            qT.append((qa, qb))
                            kT.append(k_t)
                        with tc.tile_pool(name="wv", bufs=4) as wvpool:
                            wv_t = []
                            for cc in range(ND // 2):
                                w = wvpool.tile([128, 2048], FP8, tag="wv",
                                                name="wv_t")
                                nc.sync.dma_start(w[:], wv8[cc])
                                wv_t.append(w.rearrange(
                                    "p (two n) -> p two n", two=2))
                            # fp8 v PAIR tiles [128, 2(i-par), H, 128]: per
                            # head, 64 v-columns and 64 (1/32)-columns (ones
                            # block first for odd heads) -- the av DoubleRow
                            # matmul then emits the softmax denominator
                            # (x1/32) broadcast across 64 partitions right
                            # next to the 64 output features.
                            for a in range(NT // 2):
                                vt = vp.tile([128, 2 * H * 128], FP8,
                                             tag="v", name="vt")
                                v5 = vt.rearrange("p (two h e) -> p two h e",
                                                  two=2, e=128)
                                nc.gpsimd.memset(vt[:], 1.0 / 32.0)
                                for par in range(2):
                                    i = 2 * a + par
                                    for nb in range(2):
                                        ps = psq.tile([128, 512], F32,
                                                      tag="v", bufs=2,
                                                      name="psv")
                                        for cc in range(ND // 2):
                                            xsl = x83[cc][
                                                :, i // 4, :,
                                                128 * (i % 4):
                                                128 * (i % 4) + 128]
                                            nc.tensor.matmul(
                                                ps[:], xsl,
                                                wv_t[cc][:, :,
                                                         512 * nb:
                                                         512 * (nb + 1)],
                                                start=(cc == 0),
                                                stop=(cc == ND // 2 - 1),
                                                perf_mode=DR)
                                        p4 = ps[:].rearrange(
                                            "p (h e) -> p h e", e=64)
                                        nc.scalar.mul(
                                            v5[:, par, 8 * nb:8 * (nb + 1):2,
                                               0:64],
                                            p4[:, 0:8:2, :], 1.0 / 64.0)
                                        nc.scalar.mul(
                                            v5[:, par,
                                               8 * nb + 1:8 * (nb + 1):2,
                                               64:128],
                                            p4[:, 1:8:2, :], 1.0 / 64.0)
                                v_sb.append(vt)
                    if upto == 2:
                        for c in range(2):
                            nc.sync.dma_start(
                                out_l[128 * c:128 * (c + 1), :],
                                v32(qT[c][:]))
                            nc.sync.dma_start(
                                out_l[128 * (c + 2):128 * (c + 3), :],
                                v32(kT[c][:]))
                        raise _Done()

                    # ===== phase 3: causal attention (v-stationary av) =====
                    # av flipped: out[e,t] = v^T @ se accumulated over key
                    # blocks i straight into a [65,T] psum (row 64 = softmax
                    # denominator via the ones-column in v).  One matmul per
                    # (i, psum bank) instead of per (i, token block): large
                    # moving operands, and the output lands feature-major so
                    # the proj transposes disappear.  Odd heads reach
                    # partitions 64:128 of the pair tile via SBUF->SBUF DMA
                    # (DVE cannot shift partitions).
                    with (
                        tc.tile_pool(name="sc", bufs=10) as scp,
                        tc.tile_pool(name="dvt", bufs=1) as dvp,
                        tc.tile_pool(name="ps_b", bufs=1, space="PSUM") as psb,
                    ):
                        v5i = [v_sb[a].rearrange("p (two h e) -> p two h e",
                                                 two=2, e=128)
                               for a in range(NT // 2)]
                        SC = 0.125 / (64.0 * 64.0)
                        Us = [None] * H

                        def div_front(h):
                            # fast approx recip over ALL 128 lanes (the
                            # custom DVE op wants base partition 0; the
                            # feature-lane recips are garbage and get
                            # overwritten by the DMA lane shift below before
                            # anyone reads them), then DMA the denominator
                            # recips across the lane boundary into a
                            # separate tile
                            q = h % 2
                            fsl = slice(64 * q, 64 * (q + 1))
                            dsl = slice(64 * (1 - q), 64 * (2 - q))
                            R = dvp.tile([128, T], F32, tag="R", bufs=2,
                                         name="R")
                            nc.vector.reciprocal_approx_fast(R[:], Us[h][:])
                            R2 = dvp.tile([128, T], F32, tag="R2", bufs=2,
                                          name="R2")
                            nc.sync.dma_start(R2[fsl, :], R[dsl, :])
                            return R2

                        def div_back(h, R2):
                            q = h % 2
                            fsl = slice(64 * q, 64 * (q + 1))
                            dst = ao8[h // 4].rearrange(
                                "p (two n) -> p two n", two=2)
                            nc.vector.tensor_mul(
                                dst[fsl, (h // 2) % 2, :],
                                Us[h][fsl, :], R2[fsl, :])

                        # Two-deep software pipeline: iteration h emits the
                        # scores/exp/mask of head h with the (fp8 DoubleRow)
                        # av matmuls of head h-1 interleaved, and the
                        # softmax divide of head h-2.
                        se_hist = {}

                        def av_mm(h, a):
                            U = Us[h]
                            se3 = se_hist[h][a]
                            vsl = v5i[a][:, :, h, :]
                            lo = 256 * a
                            if lo < 512:
                                nc.tensor.matmul(
                                    U[:, lo:512], vsl,
                                    se3[:, :, lo:512],
                                    start=(a == 0), stop=(a == 1),
                                    skip_group_check=True, perf_mode=DR)
                            lo2 = max(lo, 512)
                            nc.tensor.matmul(
                                U[:, lo2:1024], vsl,
                                se3[:, :, lo2:1024],
                                start=(a == 0), stop=(a == NT // 2 - 1),
                                skip_group_check=True, perf_mode=DR)

                        dmy = psb.tile([128, T], F32, tag="sc2",
                                       bufs=2, name="dmy")
                        for _ in range(9):
                            nc.tensor.matmul(
                                dmy[:, 0:384], qT[0][0][:, 0:128],
                                qT[0][0][:, 0:384], start=True, stop=True)
                        Rprev = None
                        for h in range(H):
                            p, q = h // 2, h % 2
                            qsl = slice(64 * q, 64 * (q + 1))
                            if h > 1:
                                Rprev = div_front(h - 2)
                            if h > 0:
                                Us[h - 1] = psb.tile([128, T], F32,
                                                     tag="U", bufs=2,
                                                     name="U")
                            se = []
                            for i in range(NT):
                                a, par = i // 2, i % 2
                                if par == 0:
                                    st = scp.tile([128, 2 * T], FP8,
                                                  tag="sc", name="se")
                                    se3 = st.rearrange(
                                        "p (two n) -> p two n", two=2)
                                    se.append(se3)
                                    # zero the non-causal strip of the odd
                                    # sub-tile that the DoubleRow common
                                    # range will read
                                    nc.gpsimd.memset(
                                        se3[:, 1, 256 * a:256 * a + 128],
                                        0.0)
                                se3 = se[a]
                                sc2 = psb.tile([128, T], F32, tag="sc2",
                                               bufs=2, name="sc2")
                                kblk = kT[p][:, 128 * i:128 * (i + 1)]
                                qz = qT[p][q]
                                if i < 4:
                                    nc.tensor.matmul(
                                        sc2[:, 128 * i:512], kblk,
                                        qz[:, 128 * i:512],
                                        start=True, stop=True)
                                lo2 = max(512, 128 * i)
                                nc.tensor.matmul(
                                    sc2[:, lo2:1024], kblk,
                                    qz[:, lo2:1024],
                                    start=True, stop=True)
                                nc.scalar.activation(
                                    se3[:, par, 128 * i:1024],
                                    sc2[:, 128 * i:1024],
                                    AF.Exp, scale=SC)
                                dg = slice(128 * i, 128 * (i + 1))
                                nc.gpsimd.tensor_mul(
                                    se3[:, par, dg], se3[:, par, dg],
                                    c_mb[:])
                                if h > 0 and par == 1:
                                    av_mm(h - 1, a)
                            se_hist[h] = se
                            if h > 1:
                                div_back(h - 2, Rprev)
                        # drain: av + divide for the last heads
                        Us[H - 1] = psb.tile([128, T], F32, tag="U",
                                             bufs=2, name="U")
                        R = div_front(H - 2)
                        for a in range(NT // 2):
                            av_mm(H - 1, a)
                        div_back(H - 2, R)
                        R = div_front(H - 1)
                        div_back(H - 1, R)
                  if upto == 3:
                      dump_fm(aoT, ND)
                      raise _Done()

                  # ==== phase 4: projection + residual, LN2 stats fused ====
                  # LN2 stats land [128, 512] (all-ones/D stationary
                  # broadcasts the token sums across every partition at the
                  # same matmul cost) so every row op downstream runs on
                  # full DVE/ACT lanes; be2 is folded into b1'/b2' on the
                  # host so the apply is two tensor ops per half-block.
                  with tc.tile_pool(name="rw5", bufs=1) as rw5:
                   mu_b, rc_b = [], []
                   with tc.tile_pool(name="ps_st", bufs=1,
                                     space="PSUM") as pst:
                    st_x = [pst.tile([128, 512], F32, tag=f"stx{jj}",
                                     bufs=1, name="st_x")
                            for jj in range(2)]
                    st_q = [pst.tile([128, 512], F32, tag=f"stq{jj}",
                                     bufs=1, name="st_q")
                            for jj in range(2)]
                    with (
                        tc.tile_pool(name="wpj", bufs=4) as wpjp,
                        tc.tile_pool(name="sq2", bufs=8) as sq2,
                        tc.tile_pool(name="ps_c", bufs=1,
                                     space="PSUM") as psc,
                    ):
                      wp_t = []
                      for cc in range(ND // 2):
                          w = wpjp.tile([128, 2048], FP8, tag="wpj")
                          nc.sync.dma_start(w[:], wpj8[cc])
                          wp_t.append(w.rearrange(
                              "p (co two m) -> p co two m", co=ND, two=2))
                      ao3 = [ao8[cc].rearrange("p (two n) -> p two n",
                                               two=2)
                             for cc in range(ND // 2)]
                      sq = [sq2.tile([128, T], F32R, tag="sq", name="sq")
                            for _ in range(ND)]
                      for co in range(ND):
                          for jj in range(2):
                              sl = slice(512 * jj, 512 * (jj + 1))
                              ps = psc.tile([128, 512], F32, tag="pj",
                                            bufs=2, name="pspj")
                              for cc in range(ND // 2):
                                  nc.tensor.matmul(
                                      ps[:],
                                      wp_t[cc][:, co, :, :],
                                      ao3[cc][:, :, sl],
                                      start=(cc == 0),
                                      stop=(cc == ND // 2 - 1),
                                      perf_mode=DR)
                              nc.vector.scalar_tensor_tensor(
                                  out=xln1[co][:, sl], in0=ps[:],
                                  scalar=c_bp[:, co:co + 1],
                                  in1=xln1[co][:, sl],
                                  op0=ALU.add, op1=ALU.add)
                          nc.vector.tensor_mul(sq[co][:], xln1[co][:],
                                               xln1[co][:])
                          for jj in range(2):
                              sl = slice(512 * jj, 512 * (jj + 1))
                              nc.tensor.matmul(
                                  st_x[jj][:], r32(c_onD[:]),
                                  xln1[co][:, sl],
                                  start=(co == 0), stop=(co == ND - 1))
                              nc.tensor.matmul(
                                  st_q[jj][:], r32(c_onD[:]),
                                  sq[co][:, sl],
                                  start=(co == 0), stop=(co == ND - 1))
                    x2 = xln1
                    if upto == 4:
                        dump_fm(x2, ND)
                        raise _Done()

                    # LN2 rows on full lanes (still inside the stats-psum
                    # scope; everything lands in SBUF so the psum frees
                    # before the FFN needs its banks)
                    for jj in range(2):
                        mu = rw5.tile([128, 512], F32, tag=f"mu{jj}",
                                      name="mu")
                        nc.vector.tensor_copy(mu[:], st_x[jj][:])
                        va = rw5.tile([128, 512], F32, tag=f"va{jj}",
                                      name="va")
                        nc.vector.tensor_mul(va[:], mu[:], mu[:])
                        nc.vector.tensor_sub(va[:], st_q[jj][:], va[:])
                        nc.scalar.activation(va[:], va[:], AF.Sqrt,
                                             bias=c_eps2[:])
                        rc = rw5.tile([128, 512], F32, tag=f"rc{jj}",
                                      name="rc")
                        nc.vector.reciprocal(rc[:], va[:])
                        mu_b.append(mu)
                        rc_b.append(rc)

                   # ==== phase 5: LN2 apply + xp2/xb2 ====
                   with (
                        tc.tile_pool(name="fmp", bufs=8) as fmp,
                        tc.tile_pool(name="xb2p", bufs=1) as xb2p,
                   ):
                      xb2 = [xb2p.tile([128, T], BF16, tag="xb2", bufs=8,
                                       name="xb2") for _ in range(ND)]
                      xp2 = [fmp.tile([128, D], F32, tag="xp2", name="xp2")
                             for _ in range(NT)]
                      # apply, half-block granularity so ff1 can start
                      # after the first 8 ops; sub on Pool, scale on DVE
                      for jj in range(2):
                            sl = slice(512 * jj, 512 * (jj + 1))
                            for c in range(ND):
                                nc.gpsimd.tensor_sub(
                                    x2[c][:, sl], x2[c][:, sl],
                                    mu_b[jj][:])
                                nc.vector.scalar_tensor_tensor(
                                    out=xb2[c][:, sl], in0=x2[c][:, sl],
                                    scalar=c_g2[:, c:c + 1],
                                    in1=rc_b[jj][:],
                                    op0=ALU.mult, op1=ALU.mult)
                      if upto == 5:
                            dump_fm(xb2, ND)
                            raise _Done()

                      # token-major xln2 (+be2+b2) residual ledger from the
                      # bf16 xb2 copy; 4 transposes share one psum bank
                      with tc.tile_pool(name="ps_e", bufs=1,
                                        space="PSUM") as pse:
                        for m in range(NT):
                            for ch in range(2):
                                pt = pse.tile([128, 512], BF16, tag="tr3",
                                              bufs=2, name="pt3")
                                for k in range(4):
                                    c = 4 * ch + k
                                    nc.tensor.transpose(
                                        pt[:, 128 * k:128 * (k + 1)],
                                        xb2[c][:, 128 * m:128 * (m + 1)],
                                        c_idb[:])
                                sl = slice(512 * ch, 512 * (ch + 1))
                                nc.vector.tensor_add(xp2[m][:, sl], pt[:],
                                                     c_b2[:, sl])

                    # ====== phase 6: FFN (bf16, w1 rotating prefetch) ======
                    with (
                        tc.tile_pool(name="hj", bufs=16) as hjp,
                        tc.tile_pool(name="w2t", bufs=16) as w2p,
                        tc.tile_pool(name="ps_d", bufs=1,
                                     space="PSUM") as psd,
                    ):
                        hjs = [[hjp.tile([128, T], BF16, tag="hj",
                                         name="hj") for _ in range(8)]
                               for _ in range(2)]
                        w2s = [[w2p.tile([128, D], BF16, tag="w2",
                                         name="w2t") for _ in range(8)]
                               for _ in range(2)]
                        for jg in range(4):
                            hj = hjs[jg % 2]
                            w2_t = w2s[jg % 2]
                            for j8 in range(8):
                                j = 8 * jg + j8
                                w1c = w1q[j]
                                f1 = psd.tile([128, T], F32, tag="f1",
                                              bufs=2, name="f1")
                                for jj in range(2):
                                    sl = slice(512 * jj, 512 * (jj + 1))
                                    for c in range(ND):
                                        nc.tensor.matmul(
                                            f1[:, sl],
                                            w1c[:, 128 * c:128 * (c + 1)],
                                            xb2[c][:, sl],
                                            start=(c == 0),
                                            stop=(c == ND - 1))
                                nc.scalar.activation(
                                    hj[j8][:], f1[:], AF.Relu,
                                    bias=c_b1[:, j:j + 1])
                                if j + 8 < NJ:
                                    t = w1sp.tile([128, D], BF16, tag="w1",
                                                  name="w1t")
                                    nc.sync.dma_start(
                                        t[:], w1r[j + 8].rearrange(
                                            "a b c -> a (b c)"))
                                    w1q.append(t)
                                nc.sync.dma_start(
                                    w2_t[j8][:],
                                    w2b[128 * j:128 * (j + 1), :])
                            for m in range(NT):
                                for nb in range(2):
                                    sl = slice(512 * nb, 512 * (nb + 1))
                                    fb = psd.tile([128, 512], F32,
                                                  tag="fb", bufs=4,
                                                  name="fb")
                                    for j8 in range(8):
                                        nc.tensor.matmul(
                                            fb[:],
                                            hj[j8][:,
                                                   128 * m:128 * (m + 1)],
                                            w2_t[j8][:, sl],
                                            start=(j8 == 0),
                                            stop=(j8 == 7))
                                    nc.vector.tensor_add(
                                        xp2[m][:, sl], fb[:],
                                        xp2[m][:, sl])
                        for m in range(NT):
                            nc.sync.dma_start(
                                out_l[128 * m:128 * (m + 1), :],
                                xp2[m][:])
          except _Done:
            pass

    nc.compile()
    return nc


_NC = None


def _get_nc():
    global _NC
    if _NC is None:
        _NC = _build()
    return _NC


def _prep_common(wq, wk, wv, w_proj, b_proj, w1, b1, w2, b2, g1, be1, g2, be2):
    f = np.float32
    bf = ml_dtypes.bfloat16
    fp8 = ml_dtypes.float8_e4m3fn
    wq = np.asarray(wq, f)
    wk = np.asarray(wk, f)
    wv = np.asarray(wv, f)

    def pack_qk8(w):
        # [H, D, E] -> [H/2, ND/2(cc), 128 d_in, (2 par, 2 h, 64 e)], x64
        a = (w * 64.0).reshape(H // 2, 2, ND // 2, 2, 128, E)
        a = a.transpose(0, 4, 2, 3, 1, 5)
        return np.ascontiguousarray(
            a.reshape(H // 2, 128, 1024).astype(fp8))

    # wv: [H, D, E] -> [d, (h e)] -> [cc, 128 d_in, (2 par, 1024 he)], x64
    wvm = (wv * 64.0).transpose(1, 0, 2).reshape(D, D)
    wv8 = wvm.reshape(ND // 2, 2, 128, D).transpose(0, 2, 1, 3)
    w1 = np.asarray(w1, f)
    # w_proj in fp8 (x64), packed for DoubleRow: [cc, 128 f_in, co, par, m]
    wp8 = (np.asarray(w_proj, f) * 64.0).reshape(ND // 2, 2, 128, ND, 128)
    wp8 = wp8.transpose(0, 2, 3, 1, 4).reshape(ND // 2, 128, 2048)
    return {
        "wq8": pack_qk8(wq),
        "wk8": pack_qk8(wk),
        "wv8": np.ascontiguousarray(
            wv8.reshape(ND // 2, 128, 2048).astype(fp8)),
        "wpj8": np.ascontiguousarray(wp8.astype(fp8)),
        "w1r": np.ascontiguousarray(
            w1.reshape(ND, 128, NJ, 128).transpose(2, 1, 0, 3).astype(bf)),
        "w2b": np.ascontiguousarray(np.asarray(w2, f).astype(bf)),
        "g1f": np.ascontiguousarray(
            np.asarray(g1, f).reshape(ND, 128).T * RSCL),
        "be1f": np.ascontiguousarray(
            np.asarray(be1, f).reshape(ND, 128).T * RSCL),
        "g2f": np.ascontiguousarray(np.asarray(g2, f).reshape(ND, 128).T),
        "be2f": np.ascontiguousarray(np.asarray(be2, f).reshape(ND, 128).T),
        "bpf": np.ascontiguousarray(
            np.asarray(b_proj, f).reshape(ND, 128).T * RSCL),
        "b1f": np.ascontiguousarray(
            (np.asarray(b1, f) + np.asarray(be2, f) @ w1)
            .reshape(NJ, 128).T),
        "b2b": np.ascontiguousarray(
            np.tile(np.asarray(b2, f) + np.asarray(be2, f), (128, 1))),
        "idn": np.eye(128, dtype=f),
        "onz": np.ones((128, 128), f),
        "ond": np.full((128, 1), 1.0 / D, f),
        "mby": np.where(np.arange(128)[None, :] >= np.arange(128)[:, None],
                        1.0, 0.0).astype(fp8),
    }


def kernel(x, wq, wk, wv, w_proj, b_proj, w1, b1, w2, b2, g1, be1, g2, be2,
           **bench):
    nc = _get_nc()
    common = _prep_common(wq, wk, wv, w_proj, b_proj, w1, b1, w2, b2,
                          g1, be1, g2, be2)
    x = np.asarray(x, np.float32)
    in_maps = [dict(common, x_l=np.ascontiguousarray(x[b]))
               for b in range(NCORE)]
    res = bass_utils.run_bass_kernel_spmd(
        nc, in_maps, core_ids=list(range(NCORE)), **bench)
    out = np.stack([res.results[b]["out_l"] for b in range(NCORE)])
    if bench:
        kernel.last_results = res
    return out


if __name__ == "__main__":
    _build()
    print("built ok")



# revision 66
# speedup vs baseline: 1.0096x; 1.0096x over previous
"""Trainium2 Bass kernel for a transformer block (LN1->MHA->+res->LN2->FFN->+res).

Sharding: data-parallel over batch. B=8 batch elements == 8 NeuronCores; each
core runs the whole block for one batch element (no collectives).

Per-core dataflow (T=1024, D=1024, 16 heads x 64), tuned for PE clock (HAM)
warmth -- the PE runs at 1.2 GHz until it has been busy for a full ~3.4us
window, then 2.4 GHz; every phase keeps a dense back-to-back matmul stream:
  - ~9 dummy warmup matmuls at kernel start flip the HAM throttle to 8/8
    before the first real QKV matmul
  - LN1 token-major per 128-token chunk (ACT accum_out stats); gamma/beta
    folded into the transpose-evacuation tensor_scalar; activations then
    FEATURE-major [D, T] (f32r) through the dense chain
  - causal attention per head: scoresT[s,t] block-rows land in a single
    2-bank PSUM tile [128,1024]; ONE Exp ACT per (head, s-block) covers the
    whole causal range (fuses 1/8 scale + f32->bf16); score matmuls are
    causally tightened; diagonal masked post-exp on GpSimd
  - softmax denominator from a ones-column in v (attn@[v|1]); per-partition
    reciprocal + tensor_scalar divide on DVE
  - projection in bf16 (weights streamed bf16); LN2 stats (ones-matmuls)
    interleaved with the projection matmuls, row sums copied to SBUF at proj
    end; LN2 row ops + per-feature apply feed both an f32 token-major
    residual ledger (xp2, +b2) and a bf16 copy (xb2) for ff1
  - FFN: w1/w2 streamed bf16 (w1 first set prefetched during attention,
    rotating through a 10-deep pool); relu+bias evac on the (idle) Scalar
    engine; ff2 partial sums accumulate straight into xp2 on DVE
"""

import sys

sys.path.insert(0, "/opt/trn_rl_repo")

import numpy as np
import ml_dtypes

import concourse.bacc as bacc
import concourse.mybir as mybir
from concourse.tile import TileContext
from concourse import bass_utils

F32 = mybir.dt.float32
F32R = mybir.dt.float32r
BF16 = mybir.dt.bfloat16
FP8 = mybir.dt.float8e4
AF = mybir.ActivationFunctionType
ALU = mybir.AluOpType
DR = mybir.MatmulPerfMode.DoubleRow

# xln1 is carried at RSCL x its true value (folded into gamma1) so the fp8
# proj output (attn x32 ones-fold, w_proj x64) lands on the same scale as
# the residual with zero extra ops; LN2 is scale-invariant modulo eps.
RSCL = 2048.0

B, T, D = 8, 1024, 1024
H, E = 16, 64
DFF = 4 * D
NCORE = 8
NT = T // 128
ND = D // 128
NJ = DFF // 128
LN_EPS = 1e-5


class _Done(Exception):
    pass


def r32(ap):
    return ap.bitcast(F32R)


def v32(ap):
    return ap.bitcast(F32)


def _build(upto=9):
    nc = bacc.Bacc("TRN2", target_bir_lowering=False, debug=False,
                   num_devices=NCORE)

    x_l = nc.dram_tensor("x_l", [T, D], F32R, kind="ExternalInput")
    wq8 = nc.dram_tensor("wq8", [H // 2, 128, 1024], FP8,
                         kind="ExternalInput")
    wk8 = nc.dram_tensor("wk8", [H // 2, 128, 1024], FP8,
                         kind="ExternalInput")
    wv8 = nc.dram_tensor("wv8", [ND // 2, 128, 2048], FP8,
                         kind="ExternalInput")
    wpj8 = nc.dram_tensor("wpj8", [ND // 2, 128, 2048], FP8,
                          kind="ExternalInput")
    w1r = nc.dram_tensor("w1r", [NJ, 128, ND, 128], BF16,
                         kind="ExternalInput")
    w2b = nc.dram_tensor("w2b", [DFF, D], BF16, kind="ExternalInput")
    g1f = nc.dram_tensor("g1f", [128, ND], F32, kind="ExternalInput")
    be1f = nc.dram_tensor("be1f", [128, ND], F32, kind="ExternalInput")
    g2f = nc.dram_tensor("g2f", [128, ND], F32, kind="ExternalInput")
    be2f = nc.dram_tensor("be2f", [128, ND], F32, kind="ExternalInput")
    bpf = nc.dram_tensor("bpf", [128, ND], F32, kind="ExternalInput")
    b1f = nc.dram_tensor("b1f", [128, NJ], F32, kind="ExternalInput")
    b2b = nc.dram_tensor("b2b", [128, D], F32, kind="ExternalInput")
    idn = nc.dram_tensor("idn", [128, 128], F32R, kind="ExternalInput")
    onz = nc.dram_tensor("onz", [128, 128], F32R, kind="ExternalInput")
    ond = nc.dram_tensor("ond", [128, 1], F32R, kind="ExternalInput")
    mby = nc.dram_tensor("mby", [128, 128], FP8, kind="ExternalInput")
    out_l = nc.dram_tensor("out_l", [T, D], F32, kind="ExternalOutput")

    def dump_fm(tiles, n):
        for c in range(n):
            nc.sync.dma_start(out_l[128 * c:128 * (c + 1), :],
                              v32(tiles[c][:]))

    with TileContext(nc) as tc:
        with tc.tile_pool(name="const", bufs=1) as cp:
          try:
            def cload(name, dram, shape, dtype=F32):
                t = cp.tile(list(shape), dtype, tag=name, name=name)
                nc.sync.dma_start(t[:], dram[:])
                return t

            c_id = cload("idn", idn, [128, 128], F32R)
            c_g1 = cload("g1f", g1f, [128, ND])
            c_be1 = cload("be1f", be1f, [128, ND])
            c_eps = cp.tile([128, 1], F32, tag="eps", name="eps")
            nc.gpsimd.memset(c_eps[:], LN_EPS)
            c_wrm = cp.tile([128, 384], F32, tag="wrm", name="wrm")
            nc.gpsimd.memset(c_wrm[:], 0.0)

            # ---- HAM warmup: ~4us of dummy matmuls so the PE clock is at
            # 2.4 GHz when the first real matmul issues ----
            with tc.tile_pool(name="ps_wrm", bufs=1, space="PSUM") as psw:
                for _ in range(18):
                    pw = psw.tile([128, 384], F32, tag="wrm", bufs=2,
                                  name="pwrm")
                    nc.tensor.matmul(pw[:], c_id[:], r32(c_wrm[:]),
                                     start=True, stop=True)

            with (
                tc.tile_pool(name="w1s", bufs=10) as w1sp,
                tc.tile_pool(name="fmx", bufs=8) as fmx,
                tc.tile_pool(name="rw2", bufs=1) as rw2,
            ):
                # ========== phase 1: LN1 (token-major) ==========
                xf = [fmx.tile([128, T], F32R, tag="xf", name="xf")
                      for _ in range(ND)]
                # fp8 copy of xln1 (true scale) in DoubleRow pair layout
                # [d_in, (jj, par, 512 tok)] for the fp8 QKV matmuls
                x8 = [fmx.tile([128, 2048], FP8, tag="x8", name="x8")
                      for _ in range(ND // 2)]
                with (
                    tc.tile_pool(name="xin", bufs=3) as xp,
                    tc.tile_pool(name="scr", bufs=2) as scrp,
                    tc.tile_pool(name="st1", bufs=4) as st1,
                    tc.tile_pool(name="ps_a", bufs=1, space="PSUM") as psa,
                ):
                    for m in range(NT):
                        xm = xp.tile([128, D], F32R, tag="xin")
                        nc.sync.dma_start(xm[:], x_l[128 * m:128 * (m + 1), :])
                        scr = scrp.tile([128, D], F32, tag="scr", name="scr")
                        st = st1.tile([128, 4], F32, tag="st", name="st")
                        nc.scalar.activation(scr[:], xm[:], AF.Square,
                                             accum_out=st[:, 1:2])
                        nc.scalar.activation(scr[:], xm[:], AF.Identity,
                                             accum_out=st[:, 0:1])
                        nc.vector.tensor_scalar_mul(st[:, 0:1], st[:, 0:1],
                                                    1.0 / D)
                        nc.vector.tensor_scalar_mul(st[:, 1:2], st[:, 1:2],
                                                    1.0 / D)
                        nc.vector.tensor_mul(st[:, 2:3], st[:, 0:1],
                                             st[:, 0:1])
                        nc.vector.tensor_sub(st[:, 2:3], st[:, 1:2],
                                             st[:, 2:3])
                        nc.scalar.activation(st[:, 2:3], st[:, 2:3], AF.Sqrt,
                                             bias=c_eps[:])
                        nc.vector.reciprocal(st[:, 3:4], st[:, 2:3])
                        nc.vector.tensor_scalar(
                            out=xm[:], in0=xm[:], scalar1=st[:, 0:1],
                            scalar2=st[:, 3:4], op0=ALU.subtract, op1=ALU.mult)
                        for c in range(ND):
                            pt = psa.tile([128, 128], F32R, tag="tr",
                                          bufs=3, name="pt")
                            nc.tensor.transpose(
                                pt[:], xm[:, 128 * c:128 * (c + 1)], c_id[:])
                            nc.vector.tensor_scalar(
                                out=xf[c][:, 128 * m:128 * (m + 1)],
                                in0=pt[:], scalar1=c_g1[:, c:c + 1],
                                scalar2=c_be1[:, c:c + 1],
                                op0=ALU.mult, op1=ALU.add)
                # non-critical consts load behind the x chunks
                c_idb = cp.tile([128, 128], BF16, tag="idnb", name="idnb")
                nc.vector.tensor_copy(c_idb[:], c_id[:])
                c_g2 = cload("g2f", g2f, [128, ND])
                c_be2 = cload("be2f", be2f, [128, ND])
                c_bp = cload("bpf", bpf, [128, ND])
                c_b1 = cload("b1f", b1f, [128, NJ])
                c_b2 = cload("b2b", b2b, [128, D])
                c_mb = cload("mby", mby, [128, 128], FP8)
                c_eps2 = cp.tile([128, 1], F32, tag="eps2", name="eps2")
                nc.gpsimd.memset(c_eps2[:], LN_EPS * RSCL * RSCL)
                c_onD = cp.tile([128, 128], F32, tag="onD", name="onD")
                nc.gpsimd.memset(c_onD[:], 1.0 / D)
                # x8 = xf/RSCL on ACT (idle after LN1), into the
                # DoubleRow pair layout (jj, par, tok); jj=0 first so the
                # first q/k matmuls are unblocked early
                for jj in range(2):
                    for c in range(ND):
                        x84 = x8[c // 2].rearrange(
                            "p (jj two n) -> p jj two n", jj=2, two=2)
                        nc.scalar.mul(
                            x84[:, jj, c % 2, :],
                            v32(xf[c][:, 512 * jj:512 * (jj + 1)]),
                            1.0 / RSCL)
                if upto == 1:
                    dump_fm(xf, ND)
                    raise _Done()
                xln1 = xf

                # prefetch FFN w1 set 0 (runs during attention on idle DMA)
                w1q = []
                for j8 in range(8):
                    t = w1sp.tile([128, D], BF16, tag="w1", name="w1t")
                    nc.sync.dma_start(
                        t[:], w1r[j8].rearrange("a b c -> a (b c)"))
                    w1q.append(t)

                with tc.tile_pool(name="fma", bufs=8) as fma:
                  # fp8 attention output in DoubleRow pair layout:
                  # tile cc holds feature blocks (2cc, 2cc+1) side by side
                  ao8 = [fma.tile([128, 2 * T], FP8, tag="ao", name="ao8")
                         for _ in range(ND // 2)]
                  with (
                      tc.tile_pool(name="qk", bufs=24) as qkp,
                      tc.tile_pool(name="vsb", bufs=8) as vp,
                  ):
                    # ========== phase 2: QKV + v (fp8 DoubleRow) ==========
                    qT, kT, v_sb = [], [], []
                    x83 = [x8[cc].rearrange(
                        "p (jj two n) -> p jj two n", jj=2, two=2)
                        for cc in range(ND // 2)]
                    with (
                        tc.tile_pool(name="wq", bufs=2) as wqpool,
                        tc.tile_pool(name="wk", bufs=2) as wkpool,
                        tc.tile_pool(name="ps_q", bufs=1, space="PSUM") as psq,
                    ):
                        for p in range(H // 2):
                            wq_t = wqpool.tile([128, D], FP8, tag="wq")
                            nc.sync.dma_start(
                                wq_t[:], wq8[p])
                            wk_t = wkpool.tile([128, D], FP8, tag="wk")
                            nc.sync.dma_start(
                                wk_t[:], wk8[p])
                            # q is stored TWICE, zero-padded per head, so
                            # the score matmuls can contract all 128 PE rows
                            # (the other head's k sees zeros) -- a lone
                            # 64-row matmul only lights half the array and
                            # the HAM then never grants the full clock
                            qa = qkp.tile([128, T], BF16, tag="qk",
                                          name="qa")
                            qb = qkp.tile([128, T], BF16, tag="qk",
                                          name="qb")
                            k_t = qkp.tile([128, T], BF16, tag="qk",
                                           name="k_t")
                            nc.gpsimd.memset(qa[64:128, :], 0.0)
                            nc.gpsimd.memset(qb[0:64, :], 0.0)
                            for dst, w_t in ((None, wq_t), (k_t, wk_t)):
                                w4 = w_t.rearrange(
                                    "p (cc two m) -> p cc two m",
                                    cc=ND // 2, two=2)
                                for jj in range(2):
                                    sl = slice(512 * jj, 512 * (jj + 1))
                                    ps = psq.tile([128, 512], F32, tag="qk",
                                                  bufs=3, name="psqk")
                                    for cc in range(ND // 2):
                                        nc.tensor.matmul(
                                            ps[:], w4[:, cc, :, :],
                                            x83[cc][:, jj, :, :],
                                            start=(cc == 0),
                                            stop=(cc == ND // 2 - 1),
                                            perf_mode=DR)
                                    if dst is None:
                                        nc.vector.tensor_copy(
                                            qa[0:64, sl], ps[0:64, :])
                                        nc.vector.tensor_copy(
                                            qb[64:128, sl], ps[64:128, :])
                                    else:
                                        nc.vector.tensor_copy(dst[:, sl],
                                                              ps[:])
                            qT.append((qa, qb))
                            kT.append(k_t)
                        with tc.tile_pool(name="wv", bufs=4) as wvpool:
                            wv_t = []
                            for cc in range(ND // 2):
                                w = wvpool.tile([128, 2048], FP8, tag="wv",
                                                name="wv_t")
                                nc.sync.dma_start(w[:], wv8[cc])
                                wv_t.append(w.rearrange(
                                    "p (two n) -> p two n", two=2))
                            # fp8 v PAIR tiles [128, 2(i-par), H, 128]: per
                            # head, 64 v-columns and 64 (1/32)-columns (ones
                            # block first for odd heads) -- the av DoubleRow
                            # matmul then emits the softmax denominator
                            # (x1/32) broadcast across 64 partitions right
                            # next to the 64 output features.
                            for a in range(NT // 2):
                                vt = vp.tile([128, 2 * H * 128], FP8,
                                             tag="v", name="vt")
                                v5 = vt.rearrange("p (two h e) -> p two h e",
                                                  two=2, e=128)
                                nc.gpsimd.memset(vt[:], 1.0 / 32.0)
                                for par in range(2):
                                    i = 2 * a + par
                                    for nb in range(2):
                                        ps = psq.tile([128, 512], F32,
                                                      tag="v", bufs=2,
                                                      name="psv")
                                        for cc in range(ND // 2):
                                            xsl = x83[cc][
                                                :, i // 4, :,
                                                128 * (i % 4):
                                                128 * (i % 4) + 128]
                                            nc.tensor.matmul(
                                                ps[:], xsl,
                                                wv_t[cc][:, :,
                                                         512 * nb:
                                                         512 * (nb + 1)],
                                                start=(cc == 0),
                                                stop=(cc == ND // 2 - 1),
                                                perf_mode=DR)
                                        p4 = ps[:].rearrange(
                                            "p (h e) -> p h e", e=64)
                                        nc.scalar.mul(
                                            v5[:, par, 8 * nb:8 * (nb + 1):2,
                                               0:64],
                                            p4[:, 0:8:2, :], 1.0 / 64.0)
                                        nc.scalar.mul(
                                            v5[:, par,
                                               8 * nb + 1:8 * (nb + 1):2,
                                               64:128],
                                            p4[:, 1:8:2, :], 1.0 / 64.0)
                                v_sb.append(vt)
                    if upto == 2:
                        for c in range(2):
                            nc.sync.dma_start(
                                out_l[128 * c:128 * (c + 1), :],
                                v32(qT[c][:]))
                            nc.sync.dma_start(
                                out_l[128 * (c + 2):128 * (c + 3), :],
                                v32(kT[c][:]))
                        raise _Done()

                    # ===== phase 3: causal attention (v-stationary av) =====
                    # av flipped: out[e,t] = v^T @ se accumulated over key
                    # blocks i straight into a [65,T] psum (row 64 = softmax
                    # denominator via the ones-column in v).  One matmul per
                    # (i, psum bank) instead of per (i, token block): large
                    # moving operands, and the output lands feature-major so
                    # the proj transposes disappear.  Odd heads reach
                    # partitions 64:128 of the pair tile via SBUF->SBUF DMA
                    # (DVE cannot shift partitions).
                    with (
                        tc.tile_pool(name="sc", bufs=10) as scp,
                        tc.tile_pool(name="dvt", bufs=1) as dvp,
                        tc.tile_pool(name="ps_b", bufs=1, space="PSUM") as psb,
                    ):
                        v5i = [v_sb[a].rearrange("p (two h e) -> p two h e",
                                                 two=2, e=128)
                               for a in range(NT // 2)]
                        SC = 0.125 / (64.0 * 64.0)
                        Us = [None] * H

                        def div_front(h):
                            # fast approx recip over ALL 128 lanes (the
                            # custom DVE op wants base partition 0; the
                            # feature-lane recips are garbage and get
                            # overwritten by the DMA lane shift below before
                            # anyone reads them), then DMA the denominator
                            # recips across the lane boundary into a
                            # separate tile
                            q = h % 2
                            fsl = slice(64 * q, 64 * (q + 1))
                            dsl = slice(64 * (1 - q), 64 * (2 - q))
                            R = dvp.tile([128, T], F32, tag="R", bufs=2,
                                         name="R")
                            nc.vector.reciprocal_approx_fast(R[:], Us[h][:])
                            R2 = dvp.tile([128, T], F32, tag="R2", bufs=2,
                                          name="R2")
                            nc.sync.dma_start(R2[fsl, :], R[dsl, :])
                            return R2

                        def div_back(h, R2):
                            q = h % 2
                            fsl = slice(64 * q, 64 * (q + 1))
                            dst = ao8[h // 4].rearrange(
                                "p (two n) -> p two n", two=2)
                            nc.vector.tensor_mul(
                                dst[fsl, (h // 2) % 2, :],
                                Us[h][fsl, :], R2[fsl, :])

                        # Two-deep software pipeline: iteration h emits the
                        # scores/exp/mask of head h with the (fp8 DoubleRow)
                        # av matmuls of head h-1 interleaved, and the
                        # softmax divide of head h-2.
                        se_hist = {}

                        def av_mm(h, a):
                            U = Us[h]
                            se3 = se_hist[h][a]
                            vsl = v5i[a][:, :, h, :]
                            lo = 256 * a
                            if lo < 512:
                                nc.tensor.matmul(
                                    U[:, lo:512], vsl,
                                    se3[:, :, lo:512],
                                    start=(a == 0), stop=(a == 1),
                                    skip_group_check=True, perf_mode=DR)
                            lo2 = max(lo, 512)
                            nc.tensor.matmul(
                                U[:, lo2:1024], vsl,
                                se3[:, :, lo2:1024],
                                start=(a == 0), stop=(a == NT // 2 - 1),
                                skip_group_check=True, perf_mode=DR)

                        Rprev = None
                        for h in range(H):
                            p, q = h // 2, h % 2
                            qsl = slice(64 * q, 64 * (q + 1))
                            if h > 1:
                                Rprev = div_front(h - 2)
                            if h > 0:
                                Us[h - 1] = psb.tile([128, T], F32,
                                                     tag="U", bufs=2,
                                                     name="U")
                            se = []
                            for i in range(NT):
                                a, par = i // 2, i % 2
                                if par == 0:
                                    st = scp.tile([128, 2 * T], FP8,
                                                  tag="sc", name="se")
                                    se3 = st.rearrange(
                                        "p (two n) -> p two n", two=2)
                                    se.append(se3)
                                    # zero the non-causal strip of the odd
                                    # sub-tile that the DoubleRow common
                                    # range will read
                                    nc.gpsimd.memset(
                                        se3[:, 1, 256 * a:256 * a + 128],
                                        0.0)
                                se3 = se[a]
                                sc2 = psb.tile([128, T], F32, tag="sc2",
                                               bufs=2, name="sc2")
                                kblk = kT[p][:, 128 * i:128 * (i + 1)]
                                qz = qT[p][q]
                                if i < 4:
                                    nc.tensor.matmul(
                                        sc2[:, 128 * i:512], kblk,
                                        qz[:, 128 * i:512],
                                        start=True, stop=True)
                                lo2 = max(512, 128 * i)
                                nc.tensor.matmul(
                                    sc2[:, lo2:1024], kblk,
                                    qz[:, lo2:1024],
                                    start=True, stop=True)
                                nc.scalar.activation(
                                    se3[:, par, 128 * i:1024],
                                    sc2[:, 128 * i:1024],
                                    AF.Exp, scale=SC)
                                dg = slice(128 * i, 128 * (i + 1))
                                nc.gpsimd.tensor_mul(
                                    se3[:, par, dg], se3[:, par, dg],
                                    c_mb[:])
                                if h > 0 and par == 1:
                                    av_mm(h - 1, a)
                            se_hist[h] = se
                            if h > 1:
                                div_back(h - 2, Rprev)
                        # drain: av + divide for the last heads
                        Us[H - 1] = psb.tile([128, T], F32, tag="U",
                                             bufs=2, name="U")
                        R = div_front(H - 2)
                        for a in range(NT // 2):
                            av_mm(H - 1, a)
                        div_back(H - 2, R)
                        R = div_front(H - 1)
                        div_back(H - 1, R)
                  if upto == 3:
                      dump_fm(aoT, ND)
                      raise _Done()

                  # ==== phase 4: projection + residual, LN2 stats fused ====
                  # LN2 stats land [128, 512] (all-ones/D stationary
                  # broadcasts the token sums across every partition at the
                  # same matmul cost) so every row op downstream runs on
                  # full DVE/ACT lanes; be2 is folded into b1'/b2' on the
                  # host so the apply is two tensor ops per half-block.
                  with tc.tile_pool(name="rw5", bufs=1) as rw5:
                   mu_b, rc_b = [], []
                   with tc.tile_pool(name="ps_st", bufs=1,
                                     space="PSUM") as pst:
                    st_x = [pst.tile([128, 512], F32, tag=f"stx{jj}",
                                     bufs=1, name="st_x")
                            for jj in range(2)]
                    st_q = [pst.tile([128, 512], F32, tag=f"stq{jj}",
                                     bufs=1, name="st_q")
                            for jj in range(2)]
                    with (
                        tc.tile_pool(name="wpj", bufs=4) as wpjp,
                        tc.tile_pool(name="sq2", bufs=8) as sq2,
                        tc.tile_pool(name="ps_c", bufs=1,
                                     space="PSUM") as psc,
                    ):
                      wp_t = []
                      for cc in range(ND // 2):
                          w = wpjp.tile([128, 2048], FP8, tag="wpj")
                          nc.sync.dma_start(w[:], wpj8[cc])
                          wp_t.append(w.rearrange(
                              "p (co two m) -> p co two m", co=ND, two=2))
                      ao3 = [ao8[cc].rearrange("p (two n) -> p two n",
                                               two=2)
                             for cc in range(ND // 2)]
                      sq = [sq2.tile([128, T], F32R, tag="sq", name="sq")
                            for _ in range(ND)]
                      for co in range(ND):
                          for jj in range(2):
                              sl = slice(512 * jj, 512 * (jj + 1))
                              ps = psc.tile([128, 512], F32, tag="pj",
                                            bufs=2, name="pspj")
                              for cc in range(ND // 2):
                                  nc.tensor.matmul(
                                      ps[:],
                                      wp_t[cc][:, co, :, :],
                                      ao3[cc][:, :, sl],
                                      start=(cc == 0),
                                      stop=(cc == ND // 2 - 1),
                                      perf_mode=DR)
                              nc.vector.scalar_tensor_tensor(
                                  out=xln1[co][:, sl], in0=ps[:],
                                  scalar=c_bp[:, co:co + 1],
                                  in1=xln1[co][:, sl],
                                  op0=ALU.add, op1=ALU.add)
                          nc.vector.tensor_mul(sq[co][:], xln1[co][:],
                                               xln1[co][:])
                          for jj in range(2):
                              sl = slice(512 * jj, 512 * (jj + 1))
                              nc.tensor.matmul(
                                  st_x[jj][:], r32(c_onD[:]),
                                  xln1[co][:, sl],
                                  start=(co == 0), stop=(co == ND - 1))
                              nc.tensor.matmul(
                                  st_q[jj][:], r32(c_onD[:]),
                                  sq[co][:, sl],
                                  start=(co == 0), stop=(co == ND - 1))
                    x2 = xln1
                    if upto == 4:
                        dump_fm(x2, ND)
                        raise _Done()

                    # LN2 rows on full lanes (still inside the stats-psum
                    # scope; everything lands in SBUF so the psum frees
                    # before the FFN needs its banks)
                    for jj in range(2):
                        mu = rw5.tile([128, 512], F32, tag=f"mu{jj}",
                                      name="mu")
                        nc.vector.tensor_copy(mu[:], st_x[jj][:])
                        va = rw5.tile([128, 512], F32, tag=f"va{jj}",
                                      name="va")
                        nc.vector.tensor_mul(va[:], mu[:], mu[:])
                        nc.vector.tensor_sub(va[:], st_q[jj][:], va[:])
                        nc.scalar.activation(va[:], va[:], AF.Sqrt,
                                             bias=c_eps2[:])
                        rc = rw5.tile([128, 512], F32, tag=f"rc{jj}",
                                      name="rc")
                        nc.vector.reciprocal(rc[:], va[:])
                        mu_b.append(mu)
                        rc_b.append(rc)

                   # ==== phase 5: LN2 apply + xp2/xb2 ====
                   with (
                        tc.tile_pool(name="fmp", bufs=8) as fmp,
                        tc.tile_pool(name="xb2p", bufs=1) as xb2p,
                   ):
                      xb2 = [xb2p.tile([128, T], BF16, tag="xb2", bufs=8,
                                       name="xb2") for _ in range(ND)]
                      xp2 = [fmp.tile([128, D], F32, tag="xp2", name="xp2")
                             for _ in range(NT)]
                      # apply, half-block granularity so ff1 can start
                      # after the first 8 ops; sub on Pool, scale on DVE
                      for jj in range(2):
                            sl = slice(512 * jj, 512 * (jj + 1))
                            for c in range(ND):
                                nc.gpsimd.tensor_sub(
                                    x2[c][:, sl], x2[c][:, sl],
                                    mu_b[jj][:])
                                nc.vector.scalar_tensor_tensor(
                                    out=xb2[c][:, sl], in0=x2[c][:, sl],
                                    scalar=c_g2[:, c:c + 1],
                                    in1=rc_b[jj][:],
                                    op0=ALU.mult, op1=ALU.mult)
                      if upto == 5:
                            dump_fm(xb2, ND)
                            raise _Done()

                      # token-major xln2 (+be2+b2) residual ledger from the
                      # bf16 xb2 copy; 4 transposes share one psum bank
                      with tc.tile_pool(name="ps_e", bufs=1,
                                        space="PSUM") as pse:
                        for m in range(NT):
                            for ch in range(2):
                                pt = pse.tile([128, 512], BF16, tag="tr3",
                                              bufs=2, name="pt3")
                                for k in range(4):
                                    c = 4 * ch + k
                                    nc.tensor.transpose(
                                        pt[:, 128 * k:128 * (k + 1)],
                                        xb2[c][:, 128 * m:128 * (m + 1)],
                                        c_idb[:])
                                sl = slice(512 * ch, 512 * (ch + 1))
                                nc.vector.tensor_add(xp2[m][:, sl], pt[:],
                                                     c_b2[:, sl])

                    # ====== phase 6: FFN (bf16, w1 rotating prefetch) ======
                    with (
                        tc.tile_pool(name="hj", bufs=16) as hjp,
                        tc.tile_pool(name="w2t", bufs=16) as w2p,
                        tc.tile_pool(name="ps_d", bufs=1,
                                     space="PSUM") as psd,
                    ):
                        hjs = [[hjp.tile([128, T], BF16, tag="hj",
                                         name="hj") for _ in range(8)]
                               for _ in range(2)]
                        w2s = [[w2p.tile([128, D], BF16, tag="w2",
                                         name="w2t") for _ in range(8)]
                               for _ in range(2)]
                        for jg in range(4):
                            hj = hjs[jg % 2]
                            w2_t = w2s[jg % 2]
                            for j8 in range(8):
                                j = 8 * jg + j8
                                w1c = w1q[j]
                                f1 = psd.tile([128, T], F32, tag="f1",
                                              bufs=2, name="f1")
                                for jj in range(2):
                                    sl = slice(512 * jj, 512 * (jj + 1))
                                    for c in range(ND):
                                        nc.tensor.matmul(
                                            f1[:, sl],
                                            w1c[:, 128 * c:128 * (c + 1)],
                                            xb2[c][:, sl],
                                            start=(c == 0),
                                            stop=(c == ND - 1))
                                nc.scalar.activation(
                                    hj[j8][:], f1[:], AF.Relu,
                                    bias=c_b1[:, j:j + 1])
                                if j + 8 < NJ:
                                    t = w1sp.tile([128, D], BF16, tag="w1",
                                                  name="w1t")
                                    nc.sync.dma_start(
                                        t[:], w1r[j + 8].rearrange(
                                            "a b c -> a (b c)"))
                                    w1q.append(t)
                                nc.sync.dma_start(
                                    w2_t[j8][:],
                                    w2b[128 * j:128 * (j + 1), :])
                            for m in range(NT):
                                for nb in range(2):
                                    sl = slice(512 * nb, 512 * (nb + 1))
                                    fb = psd.tile([128, 512], F32,
                                                  tag="fb", bufs=4,
                                                  name="fb")
                                    for j8 in range(8):
                                        nc.tensor.matmul(
                                            fb[:],
                                            hj[j8][:,
                                                   128 * m:128 * (m + 1)],
                                            w2_t[j8][:, sl],
                                            start=(j8 == 0),
                                            stop=(j8 == 7))
                                    nc.vector.tensor_add(
                                        xp2[m][:, sl], fb[:],
                                        xp2[m][:, sl])
                        for m in range(NT):
                            nc.sync.dma_start(
                                out_l[128 * m:128 * (m + 1), :],
                                xp2[m][:])
          except _Done:
            pass

    nc.compile()
    return nc


_NC = None


def _get_nc():
    global _NC
    if _NC is None:
        _NC = _build()
    return _NC


def _prep_common(wq, wk, wv, w_proj, b_proj, w1, b1, w2, b2, g1, be1, g2, be2):
    f = np.float32
    bf = ml_dtypes.bfloat16
    fp8 = ml_dtypes.float8_e4m3fn
    wq = np.asarray(wq, f)
    wk = np.asarray(wk, f)
    wv = np.asarray(wv, f)

    def pack_qk8(w):
        # [H, D, E] -> [H/2, ND/2(cc), 128 d_in, (2 par, 2 h, 64 e)], x64
        a = (w * 64.0).reshape(H // 2, 2, ND // 2, 2, 128, E)
        a = a.transpose(0, 4, 2, 3, 1, 5)
        return np.ascontiguousarray(
            a.reshape(H // 2, 128, 1024).astype(fp8))

    # wv: [H, D, E] -> [d, (h e)] -> [cc, 128 d_in, (2 par, 1024 he)], x64
    wvm = (wv * 64.0).transpose(1, 0, 2).reshape(D, D)
    wv8 = wvm.reshape(ND // 2, 2, 128, D).transpose(0, 2, 1, 3)
    w1 = np.asarray(w1, f)
    # w_proj in fp8 (x64), packed for DoubleRow: [cc, 128 f_in, co, par, m]
    wp8 = (np.asarray(w_proj, f) * 64.0).reshape(ND // 2, 2, 128, ND, 128)
    wp8 = wp8.transpose(0, 2, 3, 1, 4).reshape(ND // 2, 128, 2048)
    return {
        "wq8": pack_qk8(wq),
        "wk8": pack_qk8(wk),
        "wv8": np.ascontiguousarray(
            wv8.reshape(ND // 2, 128, 2048).astype(fp8)),
        "wpj8": np.ascontiguousarray(wp8.astype(fp8)),
        "w1r": np.ascontiguousarray(
            w1.reshape(ND, 128, NJ, 128).transpose(2, 1, 0, 3).astype(bf)),
        "w2b": np.ascontiguousarray(np.asarray(w2, f).astype(bf)),
        "g1f": np.ascontiguousarray(
            np.asarray(g1, f).reshape(ND, 128).T * RSCL),
        "be1f": np.ascontiguousarray(
            np.asarray(be1, f).reshape(ND, 128).T * RSCL),
        "g2f": np.ascontiguousarray(np.asarray(g2, f).reshape(ND, 128).T),
        "be2f": np.ascontiguousarray(np.asarray(be2, f).reshape(ND, 128).T),
        "bpf": np.ascontiguousarray(
            np.asarray(b_proj, f).reshape(ND, 128).T * RSCL),
        "b1f": np.ascontiguousarray(
            (np.asarray(b1, f) + np.asarray(be2, f) @ w1)
            .reshape(NJ, 128).T),
        "b2b": np.ascontiguousarray(
            np.tile(np.asarray(b2, f) + np.asarray(be2, f), (128, 1))),
        "idn": np.eye(128, dtype=f),
        "onz": np.ones((128, 128), f),
        "ond": np.full((128, 1), 1.0 / D, f),
        "mby": np.where(np.arange(128)[None, :] >= np.arange(128)[:, None],
                        1.0, 0.0).astype(fp8),
    }


def kernel(x, wq, wk, wv, w_proj, b_proj, w1, b1, w2, b2, g1, be1, g2, be2,
           **bench):
    nc = _get_nc()
    common = _prep_common(wq, wk, wv, w_proj, b_proj, w1, b1, w2, b2,
                          g1, be1, g2, be2)
    x = np.asarray(x, np.float32)
    in_maps = [dict(common, x_l=np.ascontiguousarray(x[b]))
               for b in range(NCORE)]
    res = bass_utils.run_bass_kernel_spmd(
        nc, in_maps, core_ids=list(range(NCORE)), **bench)
    out = np.stack([res.results[b]["out_l"] for b in range(NCORE)])
    if bench:
        kernel.last_results = res
    return out


if __name__ == "__main__":
    _build()
    print("built ok")

